# revision 20
# baseline (speedup 1.0000x reference)
"""Trainium2 Bass kernel for nn_EnhancedSpikingRetrievalCore.

Computation (see the reference model):
  - A gating path produces per-row top-2 renormalized expert weights.
    The "spiking attention" branch is exactly constant: the LIF input
    current is 1.0 at the top-KTOP positions of |x| (1.0 >= VTH=0.5), so
    those positions spike on every one of the T steps and
    mean(attention_gains, axis=-1) == KTOP/D exactly.  Only
    mean(temporal) varies per row.
  - Heavy compute: 8-expert MLP (D=2048 -> P=512 -> D=2048) with a
    gate-weighted combine.  The reference computes all E=8 experts
    densely, but after top-2 masking only 2 experts contribute per row.

This kernel exploits the top-2 sparsity: the host gathers rows by
selected expert into fixed-size single-expert segments, the device
computes  out_slot = gw * (relu(x @ W1[e] + b1[e]) @ W2[e])  for each
slot, and the host combines the two slots per row.  Total device work is
2*B = 16384 row-expert units instead of 8*B = 65536 (4x FLOP cut).

Work packing: 2*B row-units + padding are laid out into per-core
segments of sizes SEG_SIZES = (1024, 512, 512, 128) (2176 slots/core,
~6% padding).  Each segment is a contiguous run of slots computed with
one expert's weights (streamed from HBM per segment).  A small exact
search assigns experts to the global multiset of bins; any rows that
ever fail to pack (cannot happen for balanced routing) fall back to a
host-side numpy compute, preserving correctness.

The expert-selection path is numerically razor-thin (2nd/3rd gaps down
to ~1e-8), so gate weights/indices are computed with jnp ops mirroring
the reference bit-for-bit on the default jax platform.
"""

import numpy as np
import ml_dtypes

B, D, E, G, P, H = 8192, 2048, 8, 4, 512, 192
T, DT_LIF, TAU, VTH, VRESET = 20, 0.001, 0.02, 0.5, 0.0
DELTA0, KTOP, KROUTE, PREDW = 7.0, 32, 2, 0.1

N_CORES = 8
KC = D // 128              # contraction chunks over D (16)
PC = P // 128              # chunks over P (4)
DC = D // 512              # output column chunks (4)

SEG_SIZES = (1024, 512, 512, 128)  # slots per segment, per core
NSEG = len(SEG_SIZES)
S_TOTAL = sum(SEG_SIZES)           # 2176 slots per core
SEG_OFFS = tuple(int(x) for x in np.cumsum((0,) + SEG_SIZES[:-1]))

BF16 = ml_dtypes.bfloat16


def _gate_weights(x, Wg, bg, Wp, bp, Wgg, bgg):
    """Renormalized top-2 gate weights [B, E] and top-2 indices [B, 2],
    mirroring the reference gating ops verbatim (same jnp calls, default
    platform) so the ill-conditioned expert selection matches the oracle
    bit-for-bit."""
    import jax
    import jax.numpy as jnp

    x = jnp.asarray(x)
    dtype = x.dtype
    qm = jnp.mean(x, axis=-1)
    freqs = DELTA0 * jnp.arange(1, H + 1, dtype=dtype)
    ang = qm[:, None] * freqs[None, :]
    temporal = jnp.concatenate([jnp.cos(ang), jnp.sin(ang)], axis=-1)

    # mean over D of the spike rates is exactly KTOP/D for every row
    att_mean = jnp.full((x.shape[0],), np.float32(KTOP) / np.float32(D), dtype)
    gate_in = jnp.stack([jnp.mean(temporal, axis=-1), att_mean], axis=-1)

    gate_logits = gate_in @ jnp.asarray(Wg) + jnp.asarray(bg)
    gate_logits = gate_logits - PREDW * (gate_in @ jnp.asarray(Wp) + jnp.asarray(bp))
    group_logits = gate_in @ jnp.asarray(Wgg) + jnp.asarray(bgg)
    gmap = jax.nn.one_hot(jnp.arange(E) % G, G, dtype=dtype)
    gate_logits = gate_logits + group_logits @ gmap.T

    gate_weights = jax.nn.softmax(gate_logits, axis=-1)
    _, tidx = jax.lax.top_k(gate_weights, KROUTE)
    rows = jnp.arange(x.shape[0])[:, None]
    mask = jnp.zeros_like(gate_weights).at[rows, tidx].set(1.0)
    gated = gate_weights * mask
    gate_weights = gated / (jnp.sum(gated, axis=-1, keepdims=True) + 1e-9)
    return (np.asarray(gate_weights, dtype=np.float32),
            np.asarray(tidx, dtype=np.int64))


CLASS_SIZES = tuple(sorted(set(SEG_SIZES), reverse=True))
CLASS_CAP = tuple(SEG_SIZES.count(s) * N_CORES for s in CLASS_SIZES)


def _alloc_bins(needs):
    """Assign each expert a count of bins per class (CLASS_SIZES, global
    capacities CLASS_CAP) covering its row count.  Exact DFS over
    low-waste options; returns {e: counts-tuple} or None if infeasible."""
    ncls = len(CLASS_SIZES)
    cap = list(CLASS_CAP)
    order = sorted(needs, key=lambda e: -needs[e])
    assign = {}

    def options(n):
        opts = []

        def rec(ci, counts, rem):
            if ci == ncls - 1:
                sz = CLASS_SIZES[ci]
                t = (rem + sz - 1) // sz
                if t <= cap[ci]:
                    cnts = counts + (t,)
                    waste = sum(c * s for c, s in zip(cnts, CLASS_SIZES)) - n
                    if waste < CLASS_SIZES[0]:
                        opts.append((waste, cnts))
                return
            sz = CLASS_SIZES[ci]
            for c in range(min(cap[ci], (rem + sz - 1) // sz), -1, -1):
                rec(ci + 1, counts + (c,), max(0, rem - sz * c))

        rec(0, (), n)
        opts.sort()
        return opts

    def dfs(i):
        if i == len(order):
            return True
        e = order[i]
        for _, cnts in options(needs[e]):
            if any(c > r for c, r in zip(cnts, cap)):
                continue
            for ci, c in enumerate(cnts):
                cap[ci] -= c
            assign[e] = cnts
            if dfs(i + 1):
                return True
            for ci, c in enumerate(cnts):
                cap[ci] += c
            del assign[e]
        return False

    return assign if dfs(0) else None


def _pack(gw, tidx):
    """Lay out the (row, expert) work units into per-core segments.

    Returns (slot_rows [N_CORES, S_TOTAL] int32 w/ -1 pad,
             slot_gw   [N_CORES, S_TOTAL] float32,
             seg_expert [N_CORES, NSEG] int32 w/ -1 unused,
             pos [B, KROUTE] int64 global slot index per row (-1 if that
             unit fell back to the host),
             leftover list of (row, expert) units for host fallback)."""
    nb = gw.shape[0]
    rows_per_e = {}
    for k in range(KROUTE):
        for e in range(E):
            sel = np.nonzero(tidx[:, k] == e)[0]
            if len(sel):
                rows_per_e.setdefault(e, []).append(sel)
    rows_per_e = {e: np.concatenate(v) for e, v in rows_per_e.items()}
    needs = {e: len(v) for e, v in rows_per_e.items()}

    assign = _alloc_bins(needs)
    leftover = []
    if assign is None:
        # infeasible routing (cannot happen for near-balanced top-2):
        # everything falls back to the host compute
        pos = np.full((nb, KROUTE), -1, np.int64)
        for e, rows in rows_per_e.items():
            leftover.extend((int(r), e) for r in rows)
        return (np.full((N_CORES, S_TOTAL), -1, np.int32),
                np.zeros((N_CORES, S_TOTAL), np.float32),
                np.full((N_CORES, NSEG), -1, np.int32),
                pos, leftover)

    # bin id -> (core, seg) grouped by class size
    class_bins = {
        sz: [(c, s) for c in range(N_CORES)
             for s in range(NSEG) if SEG_SIZES[s] == sz]
        for sz in CLASS_SIZES
    }
    slot_rows = np.full((N_CORES, S_TOTAL), -1, np.int32)
    slot_gw = np.zeros((N_CORES, S_TOTAL), np.float32)
    seg_expert = np.full((N_CORES, NSEG), -1, np.int32)
    pos = np.full((nb, KROUTE), -1, np.int64)
    pos_fill = np.zeros(nb, np.int32)

    for e in sorted(assign, key=lambda e: -needs[e]):
        cnts = assign[e]
        bins = [class_bins[sz].pop(0)
                for sz, cn in zip(CLASS_SIZES, cnts) for _ in range(cn)]
        rows = rows_per_e[e]
        i = 0
        for (c, s) in bins:
            sz = SEG_SIZES[s]
            take = rows[i:i + sz]
            i += len(take)
            off = SEG_OFFS[s]
            seg_expert[c, s] = e
            slot_rows[c, off:off + len(take)] = take
            slot_gw[c, off:off + len(take)] = gw[take, e]
            gslot = c * S_TOTAL + off + np.arange(len(take))
            pos[take, pos_fill[take]] = gslot
            pos_fill[take] += 1
        if i < len(rows):  # did not fit (defensive; alloc covers needs)
            leftover.extend((int(r), e) for r in rows[i:])
    return slot_rows, slot_gw, seg_expert, pos, leftover


def _build_program(repeats=1, ldw_opt=True, w2_const=True, no_out=False,
                   ldw_a=None, ldw_b=None, out_bf16=True, out_dma_engine='gpsimd'):
    """Emit the per-core Tile program: NSEG single-expert segments, each
    computing  out[slot] = gw[slot] * relu(x[slot] @ W1[e] + b1[e]) @ W2[e]
    with the segment's weights streamed from HBM.

    Layouts are feature-major so both matmuls use native weight layouts:
      xg  [128, KC, S_TOTAL]        gathered X^T k-tiles, bf16 (const)
      w1  [NSEG, 128, KC, P]        per-segment W1 k-tiles, bf16
      w2  [NSEG, DC, 128, PC, 512]  per-segment W2 (dc,pc)-tiles, bf16
      gws [128, S_TOTAL // 128]     per-row-tile gate weights, fp32 (const)
      b1s [128, NSEG * PC]          per-segment b1 chunks, fp32 (const)
      out [S_TOTAL, D]              bf16 (fp32 with out_bf16=False)

    repeats > 1 re-emits the compute body (timing harness only).
    ldw_opt: reorder matmul loops to reuse the stationary operand across
    consecutive matmuls (fewer LDWEIGHTS).  w2_const / no_out: timing
    ablations (W2 resident in SBUF / skip output DMA)."""
    import concourse.bass as bass
    import concourse.mybir as mybir
    import concourse.tile as tile
    from concourse import bacc
    from concourse.bass import ts
    from contextlib import ExitStack

    if ldw_a is None:
        ldw_a = ldw_opt
    if ldw_b is None:
        ldw_b = ldw_opt

    f32 = mybir.dt.float32
    bf16 = mybir.dt.bfloat16
    AF = mybir.ActivationFunctionType

    nc = bacc.Bacc("TRN2", target_bir_lowering=False, debug=False,
                   num_devices=N_CORES)

    xg = nc.dram_tensor("xg", [128, KC, S_TOTAL], bf16,
                        kind="ExternalInput").ap()
    w1 = nc.dram_tensor("w1", [NSEG, 128, KC, P], bf16,
                        kind="ExternalInput").ap()
    w2 = nc.dram_tensor("w2", [NSEG, 128, DC, PC, 512], bf16,
                        kind="ExternalInput").ap()
    gws = nc.dram_tensor("gws", [128, S_TOTAL // 128], f32,
                         kind="ExternalInput").ap()
    b1s = nc.dram_tensor("b1s", [128, NSEG * PC], f32,
                         kind="ExternalInput").ap()
    out = nc.dram_tensor("out", [S_TOTAL, D], bf16 if out_bf16 else f32,
                         kind="ExternalOutput").ap()

    with tile.TileContext(nc) as tc, ExitStack() as ctx:
        const = ctx.enter_context(tc.tile_pool(name="const", bufs=1))
        w1p = ctx.enter_context(tc.tile_pool(name="w1p", bufs=2))
        w2p = ctx.enter_context(tc.tile_pool(name="w2p", bufs=2))
        hsp = ctx.enter_context(tc.tile_pool(name="hsp", bufs=2))
        stgp = ctx.enter_context(tc.tile_pool(name="stgp", bufs=4))
        maxbc = max((SR + 511) // 512 for SR in SEG_SIZES)
        npsa = (maxbc + 1) if ldw_a else 4
        psA = ctx.enter_context(tc.tile_pool(name="psA", bufs=npsa,
                                             space="PSUM"))
        psB = ctx.enter_context(tc.tile_pool(name="psB", bufs=8 - npsa,
                                             space="PSUM"))

        xg_sb = const.tile([128, KC, S_TOTAL], bf16)
        nc.sync.dma_start(out=xg_sb[:], in_=xg[:])
        gws_sb = const.tile([128, S_TOTAL // 128], f32)
        nc.sync.dma_start(out=gws_sb[:], in_=gws[:])
        b1_sb = const.tile([128, NSEG * PC], f32)
        nc.sync.dma_start(out=b1_sb[:], in_=b1s[:])
        w2c_sb = None
        if w2_const:
            w2c_sb = const.tile([128, NSEG, DC, PC, 512], bf16)
            for si in range(NSEG):
                nc.sync.dma_start(out=w2c_sb[:, si], in_=w2[si])

        for rep in range(repeats):
            for si in range(NSEG):
                off, SR = SEG_OFFS[si], SEG_SIZES[si]
                bcs = [(b, min(512, SR - b)) for b in range(0, SR, 512)]

                # ---- Phase A: hs = relu(x @ W1 + b1), [P, SR] bf16
                w1t = w1p.tile([128, KC, P], bf16, tag="w1")
                nc.sync.dma_start(out=w1t[:], in_=w1[si])
                hs = hsp.tile([128, PC, SR], bf16, tag="hs")

                def _actmul(ps, pc, b0, bn):
                    col = si * PC + pc
                    nc.scalar.activation(hs[:, pc, b0:b0 + bn], ps[:, :bn],
                                         AF.Relu,
                                         bias=b1_sb[:, col:col + 1])

                if ldw_a and len(bcs) > 1:
                    # reuse each w1 stationary chunk across all bc chunks
                    for pc in range(PC):
                        pss = [psA.tile([128, 512], f32, tag="psA",
                                         name=f"psA_{rep}_{si}_{pc}_{bi}")
                               for bi in range(len(bcs))]
                        for kc in range(KC):
                            for bi, (b0, bn) in enumerate(bcs):
                                nc.tensor.matmul(
                                    pss[bi][:, :bn],
                                    lhsT=w1t[:, kc, ts(pc, 128)],
                                    rhs=xg_sb[:, kc, off + b0:off + b0 + bn],
                                    start=(kc == 0),
                                    stop=(kc == KC - 1),
                                )
                        for bi, (b0, bn) in enumerate(bcs):
                            _actmul(pss[bi], pc, b0, bn)
                else:
                    for pc in range(PC):
                        for (b0, bn) in bcs:
                            ps = psA.tile([128, 512], f32, tag="psA")
                            for kc in range(KC):
                                nc.tensor.matmul(
                                    ps[:, :bn],
                                    lhsT=w1t[:, kc, ts(pc, 128)],
                                    rhs=xg_sb[:, kc, off + b0:off + b0 + bn],
                                    start=(kc == 0),
                                    stop=(kc == KC - 1),
                                )
                            _actmul(ps, pc, b0, bn)

                # ---- Phase B: out[slots, dc] = hs^T @ W2, PSUM-accumulated
                def _drain(ps2, bs, dc):
                    stg = stgp.tile([128, 512], bf16 if out_bf16 else f32,
                                    tag="stg")
                    tj = off // 128 + bs
                    nc.vector.tensor_scalar_mul(stg[:], ps2[:],
                                                gws_sb[:, tj:tj + 1])
                    if not no_out:
                        eng = getattr(nc, out_dma_engine)
                        eng.dma_start(
                            out=out[off + bs * 128:off + (bs + 1) * 128,
                                    ts(dc, 512)],
                            in_=stg[:])

                if w2_const:
                    w2t = w2c_sb[:, si]
                else:
                    w2t = w2p.tile([128, DC, PC, 512], bf16, tag="w2")
                    nc.sync.dma_start(out=w2t[:], in_=w2[si])
                if ldw_b:
                    # reuse each hs stationary chunk across pairs of dc
                    for bs in range(SR // 128):
                        for dc0 in range(0, DC, 2):
                            pss2 = [psB.tile([128, 512], f32, tag="psB",
                                             name=f"psB_{rep}_{si}_{bs}_{dc0}_{i}")
                                    for i in range(2)]
                            for pc in range(PC):
                                for i in range(2):
                                    nc.tensor.matmul(
                                        pss2[i][:],
                                        lhsT=hs[:, pc, ts(bs, 128)],
                                        rhs=w2t[:, dc0 + i, pc, :],
                                        start=(pc == 0),
                                        stop=(pc == PC - 1),
                                    )
                            for i in range(2):
                                _drain(pss2[i], bs, dc0 + i)
                else:
                    for dc in range(DC):
                        for bs in range(SR // 128):
                            ps2 = psB.tile([128, 512], f32, tag="psB")
                            for pc in range(PC):
                                nc.tensor.matmul(
                                    ps2[:],
                                    lhsT=hs[:, pc, ts(bs, 128)],
                                    rhs=w2t[:, dc, pc, :],
                                    start=(pc == 0),
                                    stop=(pc == PC - 1),
                                )
                            _drain(ps2, bs, dc)

    nc.compile()
    return nc


_program_cache = {}


def _get_program():
    if "nc" not in _program_cache:
        _program_cache["nc"] = _build_program()
    return _program_cache["nc"]


def _make_in_maps(inputs):
    x = np.asarray(inputs["query_embedding"], dtype=np.float32)
    W1 = np.asarray(inputs["W1"], dtype=np.float32)
    W2 = np.asarray(inputs["W2"], dtype=np.float32)
    b1 = np.asarray(inputs["b1"], dtype=np.float32)

    gw, tidx = _gate_weights(x, inputs["Wg"], inputs["bg"], inputs["Wp"],
                             inputs["bp"], inputs["Wgg"], inputs["bgg"])
    slot_rows, slot_gw, seg_expert, pos, leftover = _pack(gw, tidx)

    xb = x.astype(BF16)
    w1h = {}
    w2h = {}
    for e in np.unique(seg_expert):
        if e < 0:
            continue
        w1h[e] = np.ascontiguousarray(
            W1[e].astype(BF16).reshape(KC, 128, P).transpose(1, 0, 2))
        w2h[e] = np.ascontiguousarray(
            W2[e].astype(BF16).reshape(PC, 128, DC, 512).transpose(1, 2, 0, 3))

    in_maps = []
    for c in range(N_CORES):
        ridx = np.where(slot_rows[c] < 0, 0, slot_rows[c])
        xs = xb[ridx]                                   # [S_TOTAL, D]
        xgh = np.ascontiguousarray(
            xs.T.reshape(KC, 128, S_TOTAL).transpose(1, 0, 2))
        gwsh = np.ascontiguousarray(
            slot_gw[c].reshape(S_TOTAL // 128, 128).T.astype(np.float32))
        w1c = np.zeros((NSEG, 128, KC, P), BF16)
        w2c = np.zeros((NSEG, 128, DC, PC, 512), BF16)
        b1c = np.zeros((128, NSEG * PC), np.float32)
        for s in range(NSEG):
            e = seg_expert[c, s]
            if e < 0:
                continue
            w1c[s] = w1h[e]
            w2c[s] = w2h[e]
            b1c[:, s * PC:(s + 1) * PC] = b1[e].reshape(PC, 128).T
        in_maps.append({"xg": xgh, "w1": w1c, "w2": w2c, "gws": gwsh,
                        "b1s": b1c})
    return in_maps, (gw, pos, leftover)


def _host_fallback(x, W1, W2, b1, gw, leftover):
    """Exact-shape fp32 host compute for (row, expert) units that did not
    pack (normally none)."""
    add = np.zeros((x.shape[0], D), np.float32)
    by_e = {}
    for r, e in leftover:
        by_e.setdefault(e, []).append(r)
    for e, rows in by_e.items():
        rows = np.asarray(rows)
        xr = x[rows].astype(BF16).astype(np.float32)
        h = np.maximum(xr @ W1[e].astype(BF16).astype(np.float32) + b1[e], 0.0)
        h = (h.astype(BF16).astype(np.float32)
             * gw[rows, e:e + 1].astype(BF16).astype(np.float32))
        add[rows] += h @ W2[e].astype(BF16).astype(np.float32)
    return add


def _run(inputs, trace=False):
    from concourse.bass_utils import run_bass_kernel_spmd

    in_maps, (gw, pos, leftover) = _make_in_maps(inputs)
    b2 = np.asarray(inputs["b2"], dtype=np.float32)

    nc = _get_program()
    res = run_bass_kernel_spmd(nc, in_maps, list(range(N_CORES)), trace=trace)
    out_all = np.concatenate(
        [res.results[c]["out"] for c in range(N_CORES)],
        axis=0).astype(np.float32)
    out_all = np.concatenate(
        [out_all, np.zeros((1, D), np.float32)], axis=0)  # slot -1 -> 0

    full = out_all[pos[:, 0]] + out_all[pos[:, 1]]
    if leftover:
        x = np.asarray(inputs["query_embedding"], dtype=np.float32)
        full = full + _host_fallback(
            x, np.asarray(inputs["W1"], np.float32),
            np.asarray(inputs["W2"], np.float32),
            np.asarray(inputs["b1"], np.float32), gw, leftover)
    if np.any(b2):
        full = full + gw @ b2
    return full.astype(np.float32), res


def kernel(**inputs) -> np.ndarray:
    out, _ = _run(inputs, trace=False)
    return out


# revision 22
# speedup vs baseline: 2.8075x; 2.8075x over previous
"""Trainium2 Bass kernel for nn_EnhancedSpikingRetrievalCore.

Computation (see the reference model):
  - A gating path produces per-row top-2 renormalized expert weights.
    The "spiking attention" branch is exactly constant: the LIF input
    current is 1.0 at the top-KTOP positions of |x| (1.0 >= VTH=0.5), so
    those positions spike on every one of the T steps and
    mean(attention_gains, axis=-1) == KTOP/D exactly.  Only
    mean(temporal) varies per row.
  - Heavy compute: 8-expert MLP (D=2048 -> P=512 -> D=2048) with a
    gate-weighted combine.  The reference computes all E=8 experts
    densely, but after top-2 masking only 2 experts contribute per row.

This kernel exploits the top-2 sparsity: the host gathers rows by
selected expert into fixed-size single-expert segments, the device
computes  out_slot = gw * (relu(x @ W1[e] + b1[e]) @ W2[e])  for each
slot, and the host combines the two slots per row.  Total device work is
2*B = 16384 row-expert units instead of 8*B = 65536 (4x FLOP cut).

Work packing: 2*B row-units + padding are laid out into per-core
segments of sizes SEG_SIZES = (1024, 512, 512, 128) (2176 slots/core,
~6% padding).  Each segment is a contiguous run of slots computed with
one expert's weights (streamed from HBM per segment).  A small exact
search assigns experts to the global multiset of bins; any rows that
ever fail to pack (cannot happen for balanced routing) fall back to a
host-side numpy compute, preserving correctness.

The expert-selection path is numerically razor-thin (2nd/3rd gaps down
to ~1e-8), so gate weights/indices are computed with jnp ops mirroring
the reference bit-for-bit on the default jax platform.
"""

import numpy as np
import ml_dtypes

B, D, E, G, P, H = 8192, 2048, 8, 4, 512, 192
T, DT_LIF, TAU, VTH, VRESET = 20, 0.001, 0.02, 0.5, 0.0
DELTA0, KTOP, KROUTE, PREDW = 7.0, 32, 2, 0.1

N_CORES = 8
KC = D // 128              # contraction chunks over D (16)
PC = P // 128              # chunks over P (4)
DC = D // 512              # output column chunks (4)

SEG_SIZES = (1024, 512, 512, 128)  # slots per segment, per core
NSEG = len(SEG_SIZES)
S_TOTAL = sum(SEG_SIZES)           # 2176 slots per core
SEG_OFFS = tuple(int(x) for x in np.cumsum((0,) + SEG_SIZES[:-1]))

BF16 = ml_dtypes.bfloat16


def _gate_weights(x, Wg, bg, Wp, bp, Wgg, bgg):
    """Renormalized top-2 gate weights [B, E] and top-2 indices [B, 2],
    mirroring the reference gating ops verbatim (same jnp calls, default
    platform) so the ill-conditioned expert selection matches the oracle
    bit-for-bit."""
    import jax
    import jax.numpy as jnp

    x = jnp.asarray(x)
    dtype = x.dtype
    qm = jnp.mean(x, axis=-1)
    freqs = DELTA0 * jnp.arange(1, H + 1, dtype=dtype)
    ang = qm[:, None] * freqs[None, :]
    temporal = jnp.concatenate([jnp.cos(ang), jnp.sin(ang)], axis=-1)

    # mean over D of the spike rates is exactly KTOP/D for every row
    att_mean = jnp.full((x.shape[0],), np.float32(KTOP) / np.float32(D), dtype)
    gate_in = jnp.stack([jnp.mean(temporal, axis=-1), att_mean], axis=-1)

    gate_logits = gate_in @ jnp.asarray(Wg) + jnp.asarray(bg)
    gate_logits = gate_logits - PREDW * (gate_in @ jnp.asarray(Wp) + jnp.asarray(bp))
    group_logits = gate_in @ jnp.asarray(Wgg) + jnp.asarray(bgg)
    gmap = jax.nn.one_hot(jnp.arange(E) % G, G, dtype=dtype)
    gate_logits = gate_logits + group_logits @ gmap.T

    gate_weights = jax.nn.softmax(gate_logits, axis=-1)
    _, tidx = jax.lax.top_k(gate_weights, KROUTE)
    rows = jnp.arange(x.shape[0])[:, None]
    mask = jnp.zeros_like(gate_weights).at[rows, tidx].set(1.0)
    gated = gate_weights * mask
    gate_weights = gated / (jnp.sum(gated, axis=-1, keepdims=True) + 1e-9)
    return (np.asarray(gate_weights, dtype=np.float32),
            np.asarray(tidx, dtype=np.int64))


CLASS_SIZES = tuple(sorted(set(SEG_SIZES), reverse=True))
CLASS_CAP = tuple(SEG_SIZES.count(s) * N_CORES for s in CLASS_SIZES)


def _alloc_bins(needs):
    """Assign each expert a count of bins per class (CLASS_SIZES, global
    capacities CLASS_CAP) covering its row count.  Exact DFS over
    low-waste options; returns {e: counts-tuple} or None if infeasible."""
    ncls = len(CLASS_SIZES)
    cap = list(CLASS_CAP)
    order = sorted(needs, key=lambda e: -needs[e])
    assign = {}

    def options(n):
        opts = []

        def rec(ci, counts, rem):
            if ci == ncls - 1:
                sz = CLASS_SIZES[ci]
                t = (rem + sz - 1) // sz
                if t <= cap[ci]:
                    cnts = counts + (t,)
                    waste = sum(c * s for c, s in zip(cnts, CLASS_SIZES)) - n
                    if waste < CLASS_SIZES[0]:
                        opts.append((waste, cnts))
                return
            sz = CLASS_SIZES[ci]
            for c in range(min(cap[ci], (rem + sz - 1) // sz), -1, -1):
                rec(ci + 1, counts + (c,), max(0, rem - sz * c))

        rec(0, (), n)
        opts.sort()
        return opts

    def dfs(i):
        if i == len(order):
            return True
        e = order[i]
        for _, cnts in options(needs[e]):
            if any(c > r for c, r in zip(cnts, cap)):
                continue
            for ci, c in enumerate(cnts):
                cap[ci] -= c
            assign[e] = cnts
            if dfs(i + 1):
                return True
            for ci, c in enumerate(cnts):
                cap[ci] += c
            del assign[e]
        return False

    return assign if dfs(0) else None


def _pack(gw, tidx):
    """Lay out the (row, expert) work units into per-core segments.

    Returns (slot_rows [N_CORES, S_TOTAL] int32 w/ -1 pad,
             slot_gw   [N_CORES, S_TOTAL] float32,
             seg_expert [N_CORES, NSEG] int32 w/ -1 unused,
             pos [B, KROUTE] int64 global slot index per row (-1 if that
             unit fell back to the host),
             leftover list of (row, expert) units for host fallback)."""
    nb = gw.shape[0]
    rows_per_e = {}
    for k in range(KROUTE):
        for e in range(E):
            sel = np.nonzero(tidx[:, k] == e)[0]
            if len(sel):
                rows_per_e.setdefault(e, []).append(sel)
    rows_per_e = {e: np.concatenate(v) for e, v in rows_per_e.items()}
    needs = {e: len(v) for e, v in rows_per_e.items()}

    assign = _alloc_bins(needs)
    leftover = []
    if assign is None:
        # infeasible routing (cannot happen for near-balanced top-2):
        # everything falls back to the host compute
        pos = np.full((nb, KROUTE), -1, np.int64)
        for e, rows in rows_per_e.items():
            leftover.extend((int(r), e) for r in rows)
        return (np.full((N_CORES, S_TOTAL), -1, np.int32),
                np.zeros((N_CORES, S_TOTAL), np.float32),
                np.full((N_CORES, NSEG), -1, np.int32),
                pos, leftover)

    # bin id -> (core, seg) grouped by class size
    class_bins = {
        sz: [(c, s) for c in range(N_CORES)
             for s in range(NSEG) if SEG_SIZES[s] == sz]
        for sz in CLASS_SIZES
    }
    slot_rows = np.full((N_CORES, S_TOTAL), -1, np.int32)
    slot_gw = np.zeros((N_CORES, S_TOTAL), np.float32)
    seg_expert = np.full((N_CORES, NSEG), -1, np.int32)
    pos = np.full((nb, KROUTE), -1, np.int64)
    pos_fill = np.zeros(nb, np.int32)

    for e in sorted(assign, key=lambda e: -needs[e]):
        cnts = assign[e]
        bins = [class_bins[sz].pop(0)
                for sz, cn in zip(CLASS_SIZES, cnts) for _ in range(cn)]
        rows = rows_per_e[e]
        i = 0
        for (c, s) in bins:
            sz = SEG_SIZES[s]
            take = rows[i:i + sz]
            i += len(take)
            off = SEG_OFFS[s]
            seg_expert[c, s] = e
            slot_rows[c, off:off + len(take)] = take
            slot_gw[c, off:off + len(take)] = gw[take, e]
            gslot = c * S_TOTAL + off + np.arange(len(take))
            pos[take, pos_fill[take]] = gslot
            pos_fill[take] += 1
        if i < len(rows):  # did not fit (defensive; alloc covers needs)
            leftover.extend((int(r), e) for r in rows[i:])
    return slot_rows, slot_gw, seg_expert, pos, leftover


def _build_program(repeats=1, ldw_opt=True, w2_const=True, no_out=False,
                   ldw_a=None, ldw_b=None, out_bf16=True, out_dma_engine='gpsimd',
                   pipe=False, deep_bufs=False):
    """Emit the per-core Tile program: NSEG single-expert segments, each
    computing  out[slot] = gw[slot] * relu(x[slot] @ W1[e] + b1[e]) @ W2[e]
    with the segment's weights streamed from HBM.

    Layouts are feature-major so both matmuls use native weight layouts:
      xg  [128, KC, S_TOTAL]        gathered X^T k-tiles, bf16 (const)
      w1  [NSEG, 128, KC, P]        per-segment W1 k-tiles, bf16
      w2  [NSEG, DC, 128, PC, 512]  per-segment W2 (dc,pc)-tiles, bf16
      gws [128, S_TOTAL // 128]     per-row-tile gate weights, fp32 (const)
      b1s [128, NSEG * PC]          per-segment b1 chunks, fp32 (const)
      out [S_TOTAL, D]              bf16 (fp32 with out_bf16=False)

    repeats > 1 re-emits the compute body (timing harness only).
    ldw_opt: reorder matmul loops to reuse the stationary operand across
    consecutive matmuls (fewer LDWEIGHTS).  w2_const / no_out: timing
    ablations (W2 resident in SBUF / skip output DMA)."""
    import concourse.bass as bass
    import concourse.mybir as mybir
    import concourse.tile as tile
    from concourse import bacc
    from concourse.bass import ts
    from contextlib import ExitStack

    if ldw_a is None:
        ldw_a = ldw_opt
    if ldw_b is None:
        ldw_b = ldw_opt

    f32 = mybir.dt.float32
    bf16 = mybir.dt.bfloat16
    AF = mybir.ActivationFunctionType

    nc = bacc.Bacc("TRN2", target_bir_lowering=False, debug=False,
                   num_devices=N_CORES)

    xg = nc.dram_tensor("xg", [128, KC, S_TOTAL], bf16,
                        kind="ExternalInput").ap()
    w1 = nc.dram_tensor("w1", [NSEG, 128, KC, P], bf16,
                        kind="ExternalInput").ap()
    w2 = nc.dram_tensor("w2", [NSEG, 128, DC, PC, 512], bf16,
                        kind="ExternalInput").ap()
    gws = nc.dram_tensor("gws", [128, S_TOTAL // 128], f32,
                         kind="ExternalInput").ap()
    b1s = nc.dram_tensor("b1s", [128, NSEG * PC], f32,
                         kind="ExternalInput").ap()
    out = nc.dram_tensor("out", [S_TOTAL, D], bf16 if out_bf16 else f32,
                         kind="ExternalOutput").ap()

    with tile.TileContext(nc) as tc, ExitStack() as ctx:
        const = ctx.enter_context(tc.tile_pool(name="const", bufs=1))
        w1p = ctx.enter_context(tc.tile_pool(name="w1p", bufs=2))
        w2p = ctx.enter_context(tc.tile_pool(name="w2p", bufs=2))
        hsp = ctx.enter_context(tc.tile_pool(name="hsp", bufs=2))
        stgp = ctx.enter_context(tc.tile_pool(name="stgp", bufs=4))
        maxbc = max((SR + 511) // 512 for SR in SEG_SIZES)
        npsa = (maxbc + 1) if ldw_a else 4
        psA = ctx.enter_context(tc.tile_pool(name="psA", bufs=npsa,
                                             space="PSUM"))
        psB = ctx.enter_context(tc.tile_pool(name="psB", bufs=8 - npsa,
                                             space="PSUM"))

        xg_sb = const.tile([128, KC, S_TOTAL], bf16)
        nc.sync.dma_start(out=xg_sb[:], in_=xg[:])
        gws_sb = const.tile([128, S_TOTAL // 128], f32)
        nc.sync.dma_start(out=gws_sb[:], in_=gws[:])
        b1_sb = const.tile([128, NSEG * PC], f32)
        nc.sync.dma_start(out=b1_sb[:], in_=b1s[:])
        w2c_sb = None
        if w2_const:
            w2c_sb = const.tile([128, NSEG, DC, PC, 512], bf16)
            for si in range(NSEG):
                nc.sync.dma_start(out=w2c_sb[:, si], in_=w2[si])

        def emit_A(rep, si):
            off, SR = SEG_OFFS[si], SEG_SIZES[si]
            bcs = [(b, min(512, SR - b)) for b in range(0, SR, 512)]
            w1t = w1p.tile([128, KC, P], bf16, tag="w1",
                           name=f"w1t_{rep}_{si}")
            nc.sync.dma_start(out=w1t[:], in_=w1[si])
            hs = hsp.tile([128, PC, SR], bf16, tag="hs",
                          name=f"hs_{rep}_{si}")

            def _act(ps, pc, b0, bn):
                col = si * PC + pc
                nc.scalar.activation(hs[:, pc, b0:b0 + bn], ps[:, :bn],
                                     AF.Relu, bias=b1_sb[:, col:col + 1])

            if ldw_a and len(bcs) > 1:
                for pc in range(PC):
                    pss = [psA.tile([128, 512], f32, tag="psA",
                                    name=f"psA_{rep}_{si}_{pc}_{bi}")
                           for bi in range(len(bcs))]
                    for kc in range(KC):
                        for bi, (b0, bn) in enumerate(bcs):
                            nc.tensor.matmul(
                                pss[bi][:, :bn],
                                lhsT=w1t[:, kc, ts(pc, 128)],
                                rhs=xg_sb[:, kc, off + b0:off + b0 + bn],
                                start=(kc == 0),
                                stop=(kc == KC - 1),
                            )
                    for bi, (b0, bn) in enumerate(bcs):
                        _act(pss[bi], pc, b0, bn)
            else:
                for pc in range(PC):
                    for (b0, bn) in bcs:
                        ps = psA.tile([128, 512], f32, tag="psA",
                                      name=f"psAn_{rep}_{si}_{pc}_{b0}")
                        for kc in range(KC):
                            nc.tensor.matmul(
                                ps[:, :bn],
                                lhsT=w1t[:, kc, ts(pc, 128)],
                                rhs=xg_sb[:, kc, off + b0:off + b0 + bn],
                                start=(kc == 0),
                                stop=(kc == KC - 1),
                            )
                        _act(ps, pc, b0, bn)
            return hs

        def emit_B(rep, si, hs):
            off, SR = SEG_OFFS[si], SEG_SIZES[si]

            def _drain(ps2, bs, dc):
                stg = stgp.tile([128, 512], bf16 if out_bf16 else f32,
                                tag="stg", name=f"stg_{rep}_{si}_{bs}_{dc}")
                tj = off // 128 + bs
                nc.vector.tensor_scalar_mul(stg[:], ps2[:],
                                            gws_sb[:, tj:tj + 1])
                if not no_out:
                    eng = getattr(nc, out_dma_engine)
                    eng.dma_start(
                        out=out[off + bs * 128:off + (bs + 1) * 128,
                                ts(dc, 512)],
                        in_=stg[:])

            if w2_const:
                w2t = w2c_sb[:, si]
            else:
                w2t = w2p.tile([128, DC, PC, 512], bf16, tag="w2",
                               name=f"w2t_{rep}_{si}")
                nc.sync.dma_start(out=w2t[:], in_=w2[si])
            if ldw_b:
                for bs in range(SR // 128):
                    for dc0 in range(0, DC, 2):
                        pss2 = [psB.tile([128, 512], f32, tag="psB",
                                         name=f"psB_{rep}_{si}_{bs}_{dc0}_{i}")
                                for i in range(2)]
                        for pc in range(PC):
                            for i in range(2):
                                nc.tensor.matmul(
                                    pss2[i][:],
                                    lhsT=hs[:, pc, ts(bs, 128)],
                                    rhs=w2t[:, dc0 + i, pc, :],
                                    start=(pc == 0),
                                    stop=(pc == PC - 1),
                                )
                        for i in range(2):
                            _drain(pss2[i], bs, dc0 + i)
            else:
                for dc in range(DC):
                    for bs in range(SR // 128):
                        ps2 = psB.tile([128, 512], f32, tag="psB",
                                       name=f"psBn_{rep}_{si}_{bs}_{dc}")
                        for pc in range(PC):
                            nc.tensor.matmul(
                                ps2[:],
                                lhsT=hs[:, pc, ts(bs, 128)],
                                rhs=w2t[:, dc, pc, :],
                                start=(pc == 0),
                                stop=(pc == PC - 1),
                            )
                        _drain(ps2, bs, dc)

        if pipe:
            pending = None
            for rep in range(repeats):
                for si in range(NSEG):
                    hs = emit_A(rep, si)
                    if pending is not None:
                        emit_B(*pending)
                    pending = (rep, si, hs)
            emit_B(*pending)
        else:
            for rep in range(repeats):
                for si in range(NSEG):
                    hs = emit_A(rep, si)
                    emit_B(rep, si, hs)

    nc.compile()
    return nc


_program_cache = {}


def _get_program():
    if "nc" not in _program_cache:
        _program_cache["nc"] = _build_program()
    return _program_cache["nc"]


def _make_in_maps(inputs):
    x = np.asarray(inputs["query_embedding"], dtype=np.float32)
    W1 = np.asarray(inputs["W1"], dtype=np.float32)
    W2 = np.asarray(inputs["W2"], dtype=np.float32)
    b1 = np.asarray(inputs["b1"], dtype=np.float32)

    gw, tidx = _gate_weights(x, inputs["Wg"], inputs["bg"], inputs["Wp"],
                             inputs["bp"], inputs["Wgg"], inputs["bgg"])
    slot_rows, slot_gw, seg_expert, pos, leftover = _pack(gw, tidx)

    xb = x.astype(BF16)
    w1h = {}
    w2h = {}
    for e in np.unique(seg_expert):
        if e < 0:
            continue
        w1h[e] = np.ascontiguousarray(
            W1[e].astype(BF16).reshape(KC, 128, P).transpose(1, 0, 2))
        w2h[e] = np.ascontiguousarray(
            W2[e].astype(BF16).reshape(PC, 128, DC, 512).transpose(1, 2, 0, 3))

    in_maps = []
    for c in range(N_CORES):
        ridx = np.where(slot_rows[c] < 0, 0, slot_rows[c])
        xs = xb[ridx]                                   # [S_TOTAL, D]
        xgh = np.ascontiguousarray(
            xs.T.reshape(KC, 128, S_TOTAL).transpose(1, 0, 2))
        gwsh = np.ascontiguousarray(
            slot_gw[c].reshape(S_TOTAL // 128, 128).T.astype(np.float32))
        w1c = np.zeros((NSEG, 128, KC, P), BF16)
        w2c = np.zeros((NSEG, 128, DC, PC, 512), BF16)
        b1c = np.zeros((128, NSEG * PC), np.float32)
        for s in range(NSEG):
            e = seg_expert[c, s]
            if e < 0:
                continue
            w1c[s] = w1h[e]
            w2c[s] = w2h[e]
            b1c[:, s * PC:(s + 1) * PC] = b1[e].reshape(PC, 128).T
        in_maps.append({"xg": xgh, "w1": w1c, "w2": w2c, "gws": gwsh,
                        "b1s": b1c})
    return in_maps, (gw, pos, leftover)


def _host_fallback(x, W1, W2, b1, gw, leftover):
    """Exact-shape fp32 host compute for (row, expert) units that did not
    pack (normally none)."""
    add = np.zeros((x.shape[0], D), np.float32)
    by_e = {}
    for r, e in leftover:
        by_e.setdefault(e, []).append(r)
    for e, rows in by_e.items():
        rows = np.asarray(rows)
        xr = x[rows].astype(BF16).astype(np.float32)
        h = np.maximum(xr @ W1[e].astype(BF16).astype(np.float32) + b1[e], 0.0)
        h = (h.astype(BF16).astype(np.float32)
             * gw[rows, e:e + 1].astype(BF16).astype(np.float32))
        add[rows] += h @ W2[e].astype(BF16).astype(np.float32)
    return add


def _run(inputs, trace=False):
    from concourse.bass_utils import run_bass_kernel_spmd

    in_maps, (gw, pos, leftover) = _make_in_maps(inputs)
    b2 = np.asarray(inputs["b2"], dtype=np.float32)

    nc = _get_program()
    res = run_bass_kernel_spmd(nc, in_maps, list(range(N_CORES)), trace=trace)
    out_all = np.concatenate(
        [res.results[c]["out"] for c in range(N_CORES)],
        axis=0).astype(np.float32)
    out_all = np.concatenate(
        [out_all, np.zeros((1, D), np.float32)], axis=0)  # slot -1 -> 0

    full = out_all[pos[:, 0]] + out_all[pos[:, 1]]
    if leftover:
        x = np.asarray(inputs["query_embedding"], dtype=np.float32)
        full = full + _host_fallback(
            x, np.asarray(inputs["W1"], np.float32),
            np.asarray(inputs["W2"], np.float32),
            np.asarray(inputs["b1"], np.float32), gw, leftover)
    if np.any(b2):
        full = full + gw @ b2
    return full.astype(np.float32), res


def kernel(**inputs) -> np.ndarray:
    out, _ = _run(inputs, trace=False)
    return out
